# revision 1
# baseline (speedup 1.0000x reference)
"""Trainium2 Bass kernel: causal self-attention with RoPE (16 heads, B=2, S=2048, D=2048).

Sharding: 8 cores = 2 (batch, data-parallel) x 4 (head-groups of 4 heads, tensor
parallel).  Each core computes q/k/v projections for its 4 heads, RoPE, causal
attention, and a partial o_proj over its 512 columns of Wo's contraction dim.
The 4 partial [S, D] outputs per batch are summed on the host (the "all-reduce"
of the o_proj), which is part of the unshard/gather step.

Everything is hardcoded for the full problem shape; internals are parameterized
only so smaller self-tests can reuse the builder.
"""

import math

import numpy as np

# ---- problem constants ----
B, S, D = 2, 2048, 2048
NUM_HEADS, HD = 16, 128
N_CORES = 8
GROUPS = 4                  # head-groups (tensor-parallel)
H_PER_CORE = NUM_HEADS // GROUPS   # 4
E_PER_CORE = H_PER_CORE * HD       # 512

NEG_BIG = -1.0e30           # additive mask value (exp -> exactly 0 in fp32)

_CACHE = {}


# --------------------------------------------------------------------------
# host-side helpers
# --------------------------------------------------------------------------

def _rope_sin_cos(seq_len, head_dim):
    """float32, matches reference._rope_sin_cos."""
    pos = np.arange(seq_len, dtype=np.float32)
    freq_seq = np.arange(0, head_dim, 2, dtype=np.float32)
    inv_freq = (np.float32(1.0) / (np.float32(10000.0) ** (freq_seq / np.float32(head_dim)))).astype(np.float32)
    sinusoid = pos[:, None] * inv_freq[None, :]          # [S, hd/2]
    return np.sin(sinusoid).astype(np.float32), np.cos(sinusoid).astype(np.float32)


def _rope_tables(seq_len):
    """CC / SS' [128, seq_len] f32 in the quadrant-paired layout.
    CC row = cos(pair angle) at both x1 and x2 rows.
    SS' = +sin at x1 rows, -sin at x2 rows, so that
    shuffle16(ps*SS') = [-x2*sin at x1 rows ; x1*sin at x2 rows]."""
    sin, cos = _rope_sin_cos(seq_len, HD)       # [S, 64]
    cosT = cos.T                                # [64, S] pair-index order
    sinT = sin.T
    x1, x2 = _pair_pos()
    CC = np.empty((HD, seq_len), dtype=np.float32)
    SS = np.empty((HD, seq_len), dtype=np.float32)
    CC[x1] = cosT
    CC[x2] = cosT
    SS[x1] = sinT
    SS[x2] = -sinT
    return CC, SS


def _fp32r_round(a):
    """Round f32 array to the fp32r grid (11 mantissa bits, RNE)."""
    u = np.ascontiguousarray(a, dtype=np.float32).view(np.uint32)
    lsb = (u >> np.uint32(12)) & np.uint32(1)
    r = (u + np.uint32(0x7FF) + lsb) & np.uint32(0xFFFFF000)
    return r.view(np.float32)


def _deinterleave_idx():
    """Row permutation within one head: quadrant-paired so the RoPE partner
    swap is a within-quadrant rotation by 16 (DVE stream_shuffle-able).
    Quadrant q rows [q*32 .. q*32+15] = x1 of pairs 16q+i (orig dim 2t),
    rows [q*32+16 .. q*32+31] = x2 of those pairs (orig dim 2t+1)."""
    idx = np.empty(HD, dtype=np.int64)
    for q in range(4):
        t = 16 * q + np.arange(16)
        idx[q * 32:q * 32 + 16] = 2 * t
        idx[q * 32 + 16:q * 32 + 32] = 2 * t + 1
    return idx


def _pair_pos():
    """(x1_rows, x2_rows) in the deinterleaved layout, pair-index order."""
    x1 = np.concatenate([q * 32 + np.arange(16) for q in range(4)])
    x2 = x1 + 16
    return x1, x2


def _mask_tiles():
    """4 diagonal-block mask tiles [128, 512]: m[d][jr, ir] = 0 if 128*d+jr <= ir else NEG_BIG."""
    masks = np.zeros((4, 128, 512), dtype=np.float32)
    jr = np.arange(128)[:, None]
    ir = np.arange(512)[None, :]
    for d in range(4):
        masks[d] = np.where(128 * d + jr <= ir, 0.0, NEG_BIG)
    return masks


def _np_rope_apply(q, sin, cos):
    """q: [S, 128] in the quadrant-paired deinterleaved layout."""
    p1, p2 = _pair_pos()
    x1, x2 = q[:, p1], q[:, p2]
    r = np.empty_like(q)
    r[:, p1] = x1 * cos - x2 * sin
    r[:, p2] = x1 * sin + x2 * cos
    return r


def _np_core_model(xT, wq, wk, wv, wo):
    """Numpy model of what ONE core's device program computes.  Used for
    self-tests; mirrors the kernel's math exactly (deinterleaved rope layout)."""
    Dm, S_ = xT.shape
    E_ = wq.shape[1]
    H_ = E_ // HD
    x = xT.T                                   # [S, Dm]
    sin, cos = _rope_sin_cos(S_, HD)           # [S, 64]
    out = np.zeros((S_, Dm), dtype=np.float32)
    causal = np.tril(np.ones((S_, S_), dtype=bool))
    for h in range(H_):
        q = x @ wq[:, h * HD:(h + 1) * HD]     # [S, 128] deinterleaved dims
        k = x @ wk[:, h * HD:(h + 1) * HD]
        v = x @ wv[:, h * HD:(h + 1) * HD]     # natural dims
        q = _np_rope_apply(q, sin, cos)
        k = _np_rope_apply(k, sin, cos)
        s = (q @ k.T) / math.sqrt(HD)
        s = np.where(causal, s, -np.inf)
        p = np.exp(s - s.max(axis=-1, keepdims=True))
        p = p / p.sum(axis=-1, keepdims=True)
        out += (p @ v) @ wo[h * HD:(h + 1) * HD, :]
    return out


def _np_reference(x, Wq, Wk, Wv, Wo, attn_mask):
    """Full-problem numpy fallback replicating reference.py (generic mask)."""
    B_, S_, D_ = x.shape
    H = NUM_HEADS
    hd = D_ // H
    sin, cos = _rope_sin_cos(S_, hd)

    def proj(W):
        y = np.einsum('bsd,ed->bse', x, W)
        return y.reshape(B_, S_, H, hd).transpose(0, 2, 1, 3)

    q, k, v = proj(Wq), proj(Wk), proj(Wv)

    def rope(t):
        tr = t.reshape(B_, H, S_, hd // 2, 2)
        x1, x2 = tr[..., 0], tr[..., 1]
        r1 = x1 * cos[None, None] - x2 * sin[None, None]
        r2 = x1 * sin[None, None] + x2 * cos[None, None]
        return np.stack((r1, r2), axis=-1).reshape(B_, H, S_, hd)

    q, k = rope(q), rope(k)
    scores = np.einsum('bhqd,bhkd->bhqk', q, k) / math.sqrt(hd) + attn_mask
    scores = scores - scores.max(axis=-1, keepdims=True)
    p = np.exp(scores)
    p = p / p.sum(axis=-1, keepdims=True)
    attn = np.einsum('bhqk,bhkd->bhqd', p, v)
    attn = attn.transpose(0, 2, 1, 3).reshape(B_, S_, D_)
    return np.einsum('bsd,ed->bse', attn, Wo)


# --------------------------------------------------------------------------
# device program builder
# --------------------------------------------------------------------------

def build(S_=S, Dm_=D, H_=H_PER_CORE, mmdt="f32r"):
    """Build the per-core Bass program (SPMD: same program, 8 data shards).

    Inputs (DRAM):  xT [Dm_, S_], wq/wk/wv [Dm_, E_], wo [E_, Dm_]   (f32)
    Output (DRAM):  out [S_, Dm_] f32   (partial o_proj; host sums groups)
    """
    import concourse.bass as bass
    import concourse.tile as tile
    from concourse import bacc, mybir

    f32 = mybir.dt.float32
    bf16 = mybir.dt.bfloat16
    AF = mybir.ActivationFunctionType

    E_ = H_ * HD
    DT = Dm_ // 128            # contraction tiles
    W = min(1024, S_)          # qkv s-window
    NW = S_ // W
    NSW = W // 512             # 512-swaths per window
    NG = S_ // 512             # attention i-groups
    VB = W // 128              # v s-blocks per window
    SBK = S_ // 128            # total s-blocks
    NDB = Dm_ // 512           # o_proj D chunks
    SCALE = 1.0 / math.sqrt(HD)
    SWAP16 = [(i + 16) % 32 for i in range(32)]

    if mmdt == "f32r":
        mdt = mybir.dt.float32r
    elif mmdt == "f32":
        mdt = f32
    else:
        raise ValueError(mmdt)

    def SC(ap):  # bitcast a f32 DRAM source for DMA into an mdt tile
        return ap.bitcast(mdt) if mdt != f32 else ap

    nc = bacc.Bacc("TRN2", target_bir_lowering=False, debug=False)

    xT_d = nc.dram_tensor("xT", [Dm_, S_], f32, kind="ExternalInput")
    wqk_d = nc.dram_tensor("wqk", [Dm_, 2 * E_], f32, kind="ExternalInput")
    wv_d = nc.dram_tensor("wv", [Dm_, E_], f32, kind="ExternalInput")
    wo_d = nc.dram_tensor("wo", [E_, Dm_], f32, kind="ExternalInput")
    out_d = nc.dram_tensor("out", [S_, Dm_], f32, kind="ExternalOutput")
    v_dram = nc.dram_tensor("v_spill", [S_, E_], mdt)

    CC_np, SS_np = _rope_tables(S_)
    masks_np = _mask_tiles()
    import ml_dtypes
    cc_dram = nc.inline_tensor(CC_np, "cc_const")
    ss_dram = nc.inline_tensor(SS_np, "ss_const")
    mask_dram = nc.inline_tensor(masks_np.reshape(512, 512).astype(ml_dtypes.bfloat16), "mask_const")
    ident_dram = nc.inline_tensor(np.eye(128, dtype=ml_dtypes.bfloat16), "ident_const")
    ones_dram = nc.inline_tensor(np.ones((128, 128), dtype=np.float32), "ones_const")

    from contextlib import ExitStack

    with tile.TileContext(nc) as tc, ExitStack() as ctx:
        # ---- persistent pools (stack allocator: order matters) ----
        cpool = ctx.enter_context(tc.tile_pool(name="consts", bufs=1))
        qkpool = ctx.enter_context(tc.tile_pool(name="qkT", bufs=1))

        ident = cpool.tile([128, 128], bf16, tag="ident", name="ident")
        nc.sync.dma_start(ident[:], ident_dram[:])
        masks = []
        for dgi in range(4):
            mt = cpool.tile([128, 512], bf16, tag=f"mask{dgi}", name=f"mask{dgi}")
            nc.sync.dma_start(mt[:], mask_dram[dgi * 128:(dgi + 1) * 128, :])
            masks.append(mt)
        ones = cpool.tile([128, 128], mdt, tag="ones", name="ones")
        nc.sync.dma_start(ones[:], SC(ones_dram[:]))

        qkT = [[qkpool.tile([128, S_], mdt, tag=f"qk{h}_{p}", name=f"qk{h}_{p}") for p in range(2)]
               for h in range(H_)]
        vc0 = qkpool.tile([128, S_], mdt, tag="vc0", name="vc0")

        # ---------------- phase QKV (single pass over xT windows) ----------------
        with tc.tile_pool(name="xt", bufs=DT + 1) as xtpool, \
             tc.tile_pool(name="wvs", bufs=3) as wvpool, \
             tc.tile_pool(name="wqk", bufs=8) as wqkpool, \
             tc.tile_pool(name="rsw", bufs=5) as rswpool, \
             tc.tile_pool(name="ropew", bufs=1) as rpool, \
             tc.tile_pool(name="vstg", bufs=3) as vstg, \
             tc.tile_pool(name="qkps", bufs=8, space="PSUM") as qkvps:

            NHP = max(1, H_ // 2)     # head pairs
            for win in range(NW):
                cc = rpool.tile([128, W], f32, tag="cc", name="cc")
                nc.gpsimd.dma_start(cc[:], cc_dram[:, win * W:(win + 1) * W])
                ss = rpool.tile([128, W], f32, tag="ss", name="ss")
                nc.gpsimd.dma_start(ss[:], ss_dram[:, win * W:(win + 1) * W])
                xts = []
                for d in range(DT):
                    t = xtpool.tile([128, W], mdt, tag="xt", name="xt")
                    nc.sync.dma_start(t[:], SC(xT_d[d * 128:(d + 1) * 128, win * W:(win + 1) * W]))
                    xts.append(t)
                # v for this window -> spill to DRAM
                vps_t = [qkvps.tile([128, E_], f32, tag="psqk", name="psv") for _ in range(VB)]
                for d in range(DT):
                    wvt = wvpool.tile([128, E_], mdt, tag="wv", name="wv")
                    nc.gpsimd.dma_start(wvt[:], SC(wv_d[d * 128:(d + 1) * 128, :]))
                    for vb in range(VB):
                        nc.tensor.matmul(vps_t[vb][:], xts[d][:, vb * 128:(vb + 1) * 128],
                                         wvt[:],
                                         start=(d == 0), stop=(d == DT - 1))
                for vb in range(VB):
                    sb = win * VB + vb
                    vo = vstg.tile([128, E_], mdt, tag="vo", name="vo")
                    nc.scalar.copy(vo[:], vps_t[vb][:])
                    nc.vector.tensor_copy(vc0[:, sb * 128:(sb + 1) * 128], vps_t[vb][:, 0:128])
                    nc.gpsimd.dma_start(v_dram[sb * 128:(sb + 1) * 128, :], vo[:])
                # q/k for this window: 4 psum groups per (head-pair, q|k)
                for hp in range(NHP):
                    nh = min(2, H_ - 2 * hp)
                    for p in range(2):
                        pss = [[qkvps.tile([128, 512], f32, tag="psqk", name="psqk")
                                for _ in range(NSW)] for _ in range(nh)]
                        for d in range(DT):
                            wt = wqkpool.tile([128, 128 * nh], mdt, tag="wqk", name="wqk")
                            weng = nc.sync if d % 2 == 0 else nc.gpsimd
                            weng.dma_start(
                                wt[:], SC(wqk_d[d * 128:(d + 1) * 128,
                                                hp * 512 + p * 128 * nh:
                                                hp * 512 + (p + 1) * 128 * nh]))
                            for h2 in range(nh):
                                for sw in range(NSW):
                                    nc.tensor.matmul(pss[h2][sw][:],
                                                     wt[:, h2 * 128:(h2 + 1) * 128],
                                                     xts[d][:, sw * 512:(sw + 1) * 512],
                                                     start=(d == 0), stop=(d == DT - 1))
                        raws = []
                        for h2 in range(nh):
                            for sw in range(NSW):
                                raw = rswpool.tile([128, 512], f32, tag="raw", name="raw")
                                nc.scalar.copy(raw[:], pss[h2][sw][:])
                                raws.append((h2, sw, raw))
                        for h2, sw, raw in raws:
                            h = 2 * hp + h2
                            cw = sw * 512
                            c0 = win * W + cw
                            dst = qkT[h][p]
                            m1 = wqkpool.tile([128, 512], f32, tag="m1", name="m1", bufs=2)
                            m2 = wqkpool.tile([128, 512], f32, tag="m2", name="m2", bufs=2)
                            nc.vector.tensor_mul(m1[:], raw[:], cc[:, cw:cw + 512])
                            nc.vector.tensor_mul(m2[:], raw[:], ss[:, cw:cw + 512])
                            m2s = wqkpool.tile([128, 512], f32, tag="m2s", name="m2s", bufs=2)
                            nc.vector.stream_shuffle(m2s[:], m2[:], mask=SWAP16)
                            nc.vector.tensor_add(dst[:, c0:c0 + 512], m1[:], m2s[:])

        # ---------------- phase ATTENTION ----------------
        with tc.tile_pool(name="attnT", bufs=1) as apool, \
             tc.tile_pool(name="wo", bufs=1) as wopool:
          with tc.tile_pool(name="vcol", bufs=2) as vcpool, \
             tc.tile_pool(name="pt", bufs=6) as ptpool, \
             tc.tile_pool(name="stg", bufs=4) as stgpool, \
             tc.tile_pool(name="stps", bufs=4, space="PSUM") as stps, \
             tc.tile_pool(name="pvps", bufs=2, space="PSUM") as pvps, \
             tc.tile_pool(name="csps", bufs=2, space="PSUM") as csps:

            attnT = [apool.tile([128, S_], mdt, tag=f"attnT{h}", name=f"attnT{h}") for h in range(H_)]
            wot = [wopool.tile([128, Dm_], mdt, tag=f"wo{h}", name=f"wo{h}") for h in range(H_)]
            for h in range(H_):
                nc.gpsimd.dma_start(wot[h][:], SC(wo_d[h * 128:(h + 1) * 128, :]))

            v3 = v_dram[:].rearrange("(b p) e -> b p e", p=128)
            NCK = max(1, SBK // 4)
            for h in range(H_):
                if h == 0:
                    vc = [vc0[:, sb * 128:(sb + 1) * 128] for sb in range(SBK)]
                else:
                    # this head's v column, gathered in chunks: [p, b, c] <- [b, p, c]
                    vct = vcpool.tile([128, SBK * 128], mdt, tag="vcol", name="vcol")
                    for ck in range(4):
                        b0, b1 = ck * NCK, min((ck + 1) * NCK, SBK)
                        if b0 >= b1:
                            continue
                        nc.scalar.dma_start(
                            vct[:, b0 * 128:b1 * 128].rearrange("p (b c) -> p b c", c=128),
                            v3[b0:b1, :, h * 128:(h + 1) * 128].transpose([1, 0, 2]))
                    vc = [vct[:, sb * 128:(sb + 1) * 128] for sb in range(SBK)]
                for g in range(NG):
                    njb = 4 * g + 4
                    pv = pvps.tile([128, 512], f32, tag="pv", name="pv")
                    cs = csps.tile([128, 512], f32, tag="cs", name="cs")
                    qslice = qkT[h][0][:, g * 512:(g + 1) * 512]
                    for jb in range(njb):
                        dgi = jb - 4 * g
                        st = stps.tile([128, 512], f32, tag="st", name="st")
                        if dgi >= 0:
                            nc.tensor.matmul(st[:], ident[:], masks[dgi][:],
                                             start=True, stop=False)
                            nc.tensor.matmul(st[:], qkT[h][1][:, jb * 128:(jb + 1) * 128],
                                             qslice, start=False, stop=True)
                        else:
                            nc.tensor.matmul(st[:], qkT[h][1][:, jb * 128:(jb + 1) * 128],
                                             qslice, start=True, stop=True)
                        pt = ptpool.tile([128, 512], mdt, tag="pt", name="pt")
                        nc.scalar.activation(pt[:], st[:], AF.Exp, scale=SCALE)
                        nc.tensor.matmul(pv[:], vc[jb], pt[:],
                                         start=(jb == 0), stop=(jb == njb - 1))
                        nc.tensor.matmul(cs[:], ones[:], pt[:],
                                         start=(jb == 0), stop=(jb == njb - 1))
                    # attnT[:, group] = pv * reciprocal(cs)
                    rc = stgpool.tile([128, 512], f32, tag="rc", name="rc")
                    nc.vector.reciprocal(rc[:], cs[:])
                    nc.vector.tensor_mul(attnT[h][:, g * 512:(g + 1) * 512], pv[:], rc[:])

          # ---------------- phase O_PROJ ----------------
          with tc.tile_pool(name="ost", bufs=6) as ostpool, \
                 tc.tile_pool(name="ops", bufs=2 * NDB, space="PSUM") as opsp:
                for sb in range(SBK):
                    pss = [opsp.tile([128, 512], f32, tag="ops", name="ops") for _ in range(NDB)]
                    for h in range(H_):
                        for db in range(NDB):
                            nc.tensor.matmul(pss[db][:],
                                             attnT[h][:, sb * 128:(sb + 1) * 128],
                                             wot[h][:, db * 512:(db + 1) * 512],
                                             start=(h == 0), stop=(h == H_ - 1))
                    for db in range(NDB):
                        o = ostpool.tile([128, 512], f32, tag="ost", name="ost")
                        if db % 2 == 0:
                            nc.scalar.copy(o[:], pss[db][:])
                        else:
                            nc.vector.tensor_copy(o[:], pss[db][:])
                        nc.sync.dma_start(out_d[sb * 128:(sb + 1) * 128, db * 512:(db + 1) * 512], o[:])

    nc.compile()
    return nc


# --------------------------------------------------------------------------
# host sharding + entry point
# --------------------------------------------------------------------------

def _prep_core_inputs(x, Wq, Wk, Wv, Wo, fp32r=True):
    """Return list of 8 per-core input dicts."""
    perm = _deinterleave_idx()
    in_maps = []
    for c in range(N_CORES):
        b, g = c // GROUPS, c % GROUPS
        heads = range(g * H_PER_CORE, (g + 1) * H_PER_CORE)
        # rows of Wq/Wk for this group's heads, rope-deinterleaved within head
        qk_rows = np.concatenate([h * HD + perm for h in heads])
        v_rows = np.concatenate([np.arange(h * HD, (h + 1) * HD) for h in heads])
        rnd = _fp32r_round if fp32r else (lambda a: np.ascontiguousarray(a, dtype=np.float32))
        wq_t = Wq[qk_rows, :].T
        wk_t = Wk[qk_rows, :].T
        E_ = len(qk_rows)
        wqk = np.empty((Wq.shape[1], 2 * E_), dtype=np.float32)
        for hp in range((E_ // HD + 1) // 2):
            nh = min(2, E_ // HD - 2 * hp)
            c = 256 * hp
            wqk[:, 2 * c:2 * c + nh * 128] = wq_t[:, c:c + nh * 128]
            wqk[:, 2 * c + nh * 128:2 * c + 2 * nh * 128] = wk_t[:, c:c + nh * 128]
        in_maps.append({
            "xT": rnd(x[b].T),
            "wqk": rnd(wqk),
            "wv": rnd(Wv[v_rows, :].T),
            "wo": rnd(Wo[:, v_rows].T),
        })
    return in_maps


def _is_causal_mask(attn_mask):
    if attn_mask is None:
        return True
    m = np.asarray(attn_mask)
    if m.shape != (1, 1, S, S):
        return False
    m2 = m[0, 0]
    tril = np.tril(np.ones((S, S), dtype=bool))
    return bool(np.all(m2[tril] == 0.0) and np.all(m2[~tril] <= -1.0e30))


def _get_program(mmdt="f32r"):
    key = ("full", mmdt)
    if key not in _CACHE:
        _CACHE[key] = build(S, D, H_PER_CORE, mmdt=mmdt)
    return _CACHE[key]


def run_on_hw(in_maps, mmdt="f32r", trace=False, **kwargs):
    """Run the SPMD program on the 8 NeuronCores; returns BassKernelResults."""
    from concourse.bass_utils import run_bass_kernel_spmd
    nc = _get_program(mmdt)
    return run_bass_kernel_spmd(nc, in_maps, core_ids=list(range(N_CORES)),
                                trace=trace, **kwargs)


def kernel(x, Wq, Wk, Wv, Wo, attn_mask=None, **_ignored):
    x = np.asarray(x, dtype=np.float32)
    Wq = np.asarray(Wq, dtype=np.float32)
    Wk = np.asarray(Wk, dtype=np.float32)
    Wv = np.asarray(Wv, dtype=np.float32)
    Wo = np.asarray(Wo, dtype=np.float32)

    if not _is_causal_mask(attn_mask):
        # unexpected mask: fall back to exact host computation
        return _np_reference(x, Wq, Wk, Wv, Wo, np.asarray(attn_mask, dtype=np.float32)).astype(np.float32)

    in_maps = _prep_core_inputs(x, Wq, Wk, Wv, Wo)
    res = run_on_hw(in_maps, mmdt="f32r", trace=False)

    out = np.zeros((B, S, D), dtype=np.float32)
    for c in range(N_CORES):
        out[c // GROUPS] += res.results[c]["out"]
    return out



# revision 7
# speedup vs baseline: 1.2632x; 1.2632x over previous
"""Trainium2 Bass kernel: causal self-attention with RoPE (16 heads, B=2, S=2048, D=2048).

Sharding: 8 cores = 2 (batch, data-parallel) x 4 (head-groups of 4 heads, tensor
parallel).  Each core computes q/k/v projections for its 4 heads, RoPE, causal
attention, and a partial o_proj over its 512 rows of Wo.  The 4 partial [S, D]
outputs per batch are summed on the host (the "all-reduce" of o_proj).

All matmuls run in bf16 (cast on host), psum accumulation in f32.  Structure:
  - single x window resident in SBUF (bf16), weights streamed once
  - softmax denominator fused into the PV matmul: pt (exp scores) is the
    stationary operand, moving operand is [v | 1] (ones column baked into the
    v SBUF layout at stride 129), so the row-sum lands in psum col 128 free
  - causal diag blocks compute only the valid suffix; triangular boundary
    chunks masked post-exp on DVE
  - attention output [i, d] flipped to [d, i] via DMA-XBAR transpose
  - head-major software pipeline: attn(h) interleaved with qkv(h+1) on the
    PE queue so the tensor engine never waits on Act exp; st runs two key
    blocks ahead of pv
"""

import math

import numpy as np

# ---- problem constants ----
B, S, D = 2, 2048, 2048
NUM_HEADS, HD = 16, 128
N_CORES = 8
GROUPS = 4                  # head-groups (tensor-parallel)
H_PER_CORE = NUM_HEADS // GROUPS   # 4
E_PER_CORE = H_PER_CORE * HD       # 512

_CACHE = {}


# --------------------------------------------------------------------------
# host-side helpers
# --------------------------------------------------------------------------

def _rope_sin_cos(seq_len, head_dim):
    """float32, matches reference._rope_sin_cos."""
    pos = np.arange(seq_len, dtype=np.float32)
    freq_seq = np.arange(0, head_dim, 2, dtype=np.float32)
    inv_freq = (np.float32(1.0) / (np.float32(10000.0) ** (freq_seq / np.float32(head_dim)))).astype(np.float32)
    sinusoid = pos[:, None] * inv_freq[None, :]          # [S, hd/2]
    return np.sin(sinusoid).astype(np.float32), np.cos(sinusoid).astype(np.float32)


def _rope_tables(seq_len):
    """CC / SS' [128, seq_len] f32 in the quadrant-paired layout.
    CC row = cos(pair angle) at both x1 and x2 rows.
    SS' = +sin at x1 rows, -sin at x2 rows, so that
    shuffle16(ps*SS') = [-x2*sin at x1 rows ; x1*sin at x2 rows]."""
    sin, cos = _rope_sin_cos(seq_len, HD)       # [S, 64]
    cosT = cos.T                                # [64, S] pair-index order
    sinT = sin.T
    x1, x2 = _pair_pos()
    CC = np.empty((HD, seq_len), dtype=np.float32)
    SS = np.empty((HD, seq_len), dtype=np.float32)
    CC[x1] = cosT
    CC[x2] = cosT
    SS[x1] = sinT
    SS[x2] = -sinT
    return CC, SS


def _deinterleave_idx():
    """Row permutation within one head: quadrant-paired so the RoPE partner
    swap is a within-quadrant rotation by 16 (DVE stream_shuffle-able)."""
    idx = np.empty(HD, dtype=np.int64)
    for q in range(4):
        t = 16 * q + np.arange(16)
        idx[q * 32:q * 32 + 16] = 2 * t
        idx[q * 32 + 16:q * 32 + 32] = 2 * t + 1
    return idx


def _pair_pos():
    """(x1_rows, x2_rows) in the deinterleaved layout, pair-index order."""
    x1 = np.concatenate([q * 32 + np.arange(16) for q in range(4)])
    x2 = x1 + 16
    return x1, x2


def _np_rope_apply(q, sin, cos):
    """q: [S, 128] in the quadrant-paired deinterleaved layout."""
    p1, p2 = _pair_pos()
    x1, x2 = q[:, p1], q[:, p2]
    r = np.empty_like(q)
    r[:, p1] = x1 * cos - x2 * sin
    r[:, p2] = x1 * sin + x2 * cos
    return r


def _np_core_model(xT, wq, wk, wv, wo):
    """Numpy model of what ONE core's device program computes (f32 version
    of the math; device uses bf16)."""
    Dm, S_ = xT.shape
    E_ = wq.shape[1]
    H_ = E_ // HD
    x = xT.T.astype(np.float32)
    sin, cos = _rope_sin_cos(S_, HD)
    out = np.zeros((S_, Dm), dtype=np.float32)
    causal = np.tril(np.ones((S_, S_), dtype=bool))
    for h in range(H_):
        q = x @ wq[:, h * HD:(h + 1) * HD]
        k = x @ wk[:, h * HD:(h + 1) * HD]
        v = x @ wv[:, h * HD:(h + 1) * HD]
        q = _np_rope_apply(q, sin, cos)
        k = _np_rope_apply(k, sin, cos)
        s = (q @ k.T) / math.sqrt(HD)
        s = np.where(causal, s, -np.inf)
        p = np.exp(s - s.max(axis=-1, keepdims=True))
        p = p / p.sum(axis=-1, keepdims=True)
        out += (p @ v) @ wo[h * HD:(h + 1) * HD, :]
    return out


def _np_reference(x, Wq, Wk, Wv, Wo, attn_mask):
    """Full-problem numpy fallback replicating reference.py (generic mask)."""
    B_, S_, D_ = x.shape
    H = NUM_HEADS
    hd = D_ // H
    sin, cos = _rope_sin_cos(S_, hd)

    def proj(W):
        y = np.einsum('bsd,ed->bse', x, W)
        return y.reshape(B_, S_, H, hd).transpose(0, 2, 1, 3)

    q, k, v = proj(Wq), proj(Wk), proj(Wv)

    def rope(t):
        tr = t.reshape(B_, H, S_, hd // 2, 2)
        x1, x2 = tr[..., 0], tr[..., 1]
        r1 = x1 * cos[None, None] - x2 * sin[None, None]
        r2 = x1 * sin[None, None] + x2 * cos[None, None]
        return np.stack((r1, r2), axis=-1).reshape(B_, H, S_, hd)

    q, k = rope(q), rope(k)
    scores = np.einsum('bhqd,bhkd->bhqk', q, k) / math.sqrt(hd) + attn_mask
    scores = scores - scores.max(axis=-1, keepdims=True)
    p = np.exp(scores)
    p = p / p.sum(axis=-1, keepdims=True)
    attn = np.einsum('bhqk,bhkd->bhqd', p, v)
    attn = attn.transpose(0, 2, 1, 3).reshape(B_, S_, D_)
    return np.einsum('bsd,ed->bse', attn, Wo)


# --------------------------------------------------------------------------
# device program builder
# --------------------------------------------------------------------------

def build(S_=S, Dm_=D, H_=H_PER_CORE, mmdt="bf16"):
    """Build the per-core Bass program (SPMD: same program, 8 data shards).

    Inputs (DRAM, bf16, host pre-tiled to [128, ...] partition-major):
      xP   [128, DT*S_]        x tiles, d-major (xP[p, d*S_+s] = x[s, d*128+p])
      wqkP [128, 2*H_*DT*128]  per (h,q|k) block of DT d-tiles of [128,128]
      wvP  [128, H_*DT*128]    per h block of DT d-tiles
      woP  [128, H_*Dm_]       woP[p, h*Dm_+j] = Wo_core[h*128+p, j]
    Output (DRAM): out [S_, Dm_] bf16 (partial o_proj; host sums groups)
    """
    import concourse.tile as tile
    from concourse import bacc, mybir
    import ml_dtypes

    f32 = mybir.dt.float32
    bf16 = mybir.dt.bfloat16
    AF = mybir.ActivationFunctionType

    DT = Dm_ // 128            # contraction tiles
    NG = S_ // 512             # attention i-groups
    SBK = S_ // 128            # s-blocks
    NDB = Dm_ // 512           # o_proj D chunks
    NSW = S_ // 512            # qk projection s-swaths
    SCALE = 1.0 / math.sqrt(HD)
    SWAP16 = [(i + 16) % 32 for i in range(32)]
    VW = H_ * 129              # v sbuf tile width (ones col per head)

    nc = bacc.Bacc("TRN2", target_bir_lowering=False, debug=False)

    xP_d = nc.dram_tensor("xP", [128, DT * S_], bf16, kind="ExternalInput")
    wqkP_d = nc.dram_tensor("wqkP", [128, 2 * H_ * DT * 128], bf16, kind="ExternalInput")
    wvP_d = nc.dram_tensor("wvP", [128, H_ * DT * 128], bf16, kind="ExternalInput")
    woP_d = nc.dram_tensor("woP", [128, H_ * Dm_], bf16, kind="ExternalInput")
    out_d = nc.dram_tensor("out", [S_, Dm_], bf16, kind="ExternalOutput")

    CC_np, SS_np = _rope_tables(S_)
    cc_dram = nc.inline_tensor(CC_np.astype(ml_dtypes.bfloat16), "cc_const")
    ss_dram = nc.inline_tensor(SS_np.astype(ml_dtypes.bfloat16), "ss_const")
    tri_np = np.triu(np.ones((128, 128), dtype=np.float32)).astype(ml_dtypes.bfloat16)
    tri_dram = nc.inline_tensor(tri_np, "tri_const")

    from contextlib import ExitStack

    with tile.TileContext(nc) as tc, ExitStack() as ctx:
        # ---- persistent pools (stack allocator: order matters) ----
        cpool = ctx.enter_context(tc.tile_pool(name="consts", bufs=1))
        xpool = ctx.enter_context(tc.tile_pool(name="xt", bufs=1))
        qkpool = ctx.enter_context(tc.tile_pool(name="qkT", bufs=1))
        vpool = ctx.enter_context(tc.tile_pool(name="vsb", bufs=1))
        apool = ctx.enter_context(tc.tile_pool(name="attnT", bufs=1))
        wopool = ctx.enter_context(tc.tile_pool(name="wo", bufs=1))
        wspool = ctx.enter_context(tc.tile_pool(name="wstream", bufs=6))
        workpool = ctx.enter_context(tc.tile_pool(name="work", bufs=1))

        cc = cpool.tile([128, S_], bf16, tag="cc", name="cc")
        ss = cpool.tile([128, S_], bf16, tag="ss", name="ss")
        tri = cpool.tile([128, 128], bf16, tag="tri", name="tri")
        nc.gpsimd.dma_start(cc[:], cc_dram[:])
        nc.gpsimd.dma_start(ss[:], ss_dram[:])
        nc.gpsimd.dma_start(tri[:], tri_dram[:])

        # x tiles: split across the two HWDGE queues for startup bandwidth
        xts = []
        for d in range(DT):
            t = xpool.tile([128, S_], bf16, tag=f"x{d}", name=f"x{d}")
            eng = nc.sync if d % 2 == 0 else nc.scalar
            eng.dma_start(t[:], xP_d[:, d * S_:(d + 1) * S_])
            xts.append(t)

        qkT = [[qkpool.tile([128, S_], bf16, tag=f"qk{h}_{p}", name=f"qk{h}_{p}")
                for p in range(2)] for h in range(H_)]
        vsb = [vpool.tile([128, VW], bf16, tag=f"v{sb}", name=f"v{sb}")
               for sb in range(SBK)]
        for sb in range(SBK):       # ones columns for the fused row-sum
            nc.vector.memset(vsb[sb][:, 128::129], 1.0)

        attnT = [apool.tile([128, S_], bf16, tag=f"at{h}", name=f"at{h}")
                 for h in range(H_)]
        wot = [wopool.tile([128, Dm_], bf16, tag=f"wo{h}", name=f"wo{h}")
               for h in range(H_)]

        wq_tiles = {}

        def fetch_w(h):
            for p in range(2):
                wt = wspool.tile([128, DT * 128], bf16, tag="wst", name=f"wqk{h}{p}")
                blk = 2 * h + p
                nc.gpsimd.dma_start(wt[:], wqkP_d[:, blk * DT * 128:(blk + 1) * DT * 128])
                wq_tiles[(h, p)] = wt
            wt = wspool.tile([128, DT * 128], bf16, tag="wst", name=f"wv{h}")
            nc.gpsimd.dma_start(wt[:], wvP_d[:, h * DT * 128:(h + 1) * DT * 128])
            wq_tiles[(h, "v")] = wt

        def fetch_wo(h):
            nc.gpsimd.dma_start(wot[h][:], woP_d[:, h * Dm_:(h + 1) * Dm_])

        def wtile(shape, dtype, tag, bufs):
            return workpool.tile(shape, dtype, tag=tag, name=tag, bufs=bufs)

        with tc.tile_pool(name="stps", bufs=3, space="PSUM") as stps, \
             tc.tile_pool(name="pvps", bufs=2, space="PSUM") as pvps:

            # ---------------- emitters ----------------
            def emit_v(h, sb, qkvps):
                ps = qkvps.tile([128, 512], f32, tag="qkv", name="psv")
                wt = wq_tiles[(h, "v")]
                for d in range(DT):
                    nc.tensor.matmul(ps[:, 0:128], xts[d][:, sb * 128:(sb + 1) * 128],
                                     wt[:, d * 128:(d + 1) * 128],
                                     start=(d == 0), stop=(d == DT - 1))
                nc.vector.tensor_copy(vsb[sb][:, h * 129:h * 129 + 128], ps[:, 0:128])

            def emit_qk(h, p, sw, qkvps):
                ps = qkvps.tile([128, 512], f32, tag="qkv", name="psqk")
                wt = wq_tiles[(h, p)]
                for d in range(DT):
                    nc.tensor.matmul(ps[:], wt[:, d * 128:(d + 1) * 128],
                                     xts[d][:, sw * 512:(sw + 1) * 512],
                                     start=(d == 0), stop=(d == DT - 1))
                raw = wtile([128, 512], bf16, "raw", 3)
                nc.scalar.copy(raw[:], ps[:])
                m2 = wtile([128, 512], bf16, "m2", 2)
                nc.vector.tensor_mul(m2[:], raw[:], ss[:, sw * 512:(sw + 1) * 512])
                m2s = wtile([128, 512], bf16, "m2s", 2)
                nc.vector.stream_shuffle(m2s[:], m2[:], mask=SWAP16)
                m1 = wtile([128, 512], bf16, "m1", 2)
                nc.vector.tensor_mul(m1[:], raw[:], cc[:, sw * 512:(sw + 1) * 512])
                nc.vector.tensor_add(qkT[h][p][:, sw * 512:(sw + 1) * 512], m1[:], m2s[:])

            def qkv_closures(h, qkvps):
                """v and qk units for head h, interleaved v-between-qk."""
                vs = [lambda h=h, sb=sb: emit_v(h, sb, qkvps) for sb in range(SBK)]
                qs = [lambda h=h, p=p, sw=sw: emit_qk(h, p, sw, qkvps)
                      for p in range(2) for sw in range(NSW)]
                mixed, vi, acc = [], 0, 0.0
                ratio = len(vs) / max(1, len(qs))
                for q in qs:
                    mixed.append(q)
                    acc += ratio
                    while acc >= 1.0 and vi < len(vs):
                        mixed.append(vs[vi]); vi += 1; acc -= 1.0
                mixed.extend(vs[vi:])
                return mixed

            def attn_head(h, filler, gate=None):
                """Emit attention for head h.  `filler`: zero-arg closures
                drained between key blocks to keep the PE busy.  `gate(g)`
                limits how many fillers may run before group g completes
                (None = no gate).  st runs LOOKAHEAD key blocks ahead of pv
                so Act exp latency is hidden."""
                LOOKAHEAD = 2
                fi = [0]
                total_jb = sum(4 * g + 4 for g in range(NG))
                per_jb = len(filler) / max(1, total_jb)
                acc = [0.0]

                def drain(limit):
                    while acc[0] >= 1.0 and fi[0] < limit:
                        filler[fi[0]]()
                        fi[0] += 1
                        acc[0] -= 1.0

                for g in range(NG):
                    njb = 4 * g + 4
                    limit = len(filler) if gate is None else gate(g)
                    pva = pvps.tile([128, 258], f32, tag="pv", name="pva")
                    pvb = pvps.tile([128, 258], f32, tag="pv", name="pvb")

                    def pvc(c):
                        t = pva if c < 2 else pvb
                        return t[:, (c % 2) * 129:(c % 2) * 129 + 129]

                    pend = []   # (jb, dgi, off, pt, ptm)

                    def emit_st(jb):
                        dgi = jb - 4 * g
                        off = 128 * dgi if dgi >= 0 else 0
                        width = 512 - off
                        st = stps.tile([128, 512], f32, tag="st", name="st")
                        nc.tensor.matmul(st[:, :width],
                                         qkT[h][1][:, jb * 128:(jb + 1) * 128],
                                         qkT[h][0][:, g * 512 + off:(g + 1) * 512],
                                         start=True, stop=True)
                        pt = wtile([128, 512], bf16, "pt", 4)
                        nc.scalar.activation(pt[:, :width], st[:, :width],
                                             AF.Exp, scale=SCALE)
                        ptm = None
                        if dgi >= 0:
                            ptm = wtile([128, 128], bf16, "ptm", 3)
                            nc.vector.tensor_mul(ptm[:], pt[:, 0:128], tri[:])
                        pend.append((jb, dgi, off, pt, ptm))

                    def emit_pv():
                        # two chunks share one psum bank: only the bank's
                        # first matmul may set start (it pending-zeroes the
                        # whole 2KB zero region) and only its last sets stop
                        jb, dgi, off, pt, ptm = pend.pop(0)
                        for c in range(max(0, dgi), 4):
                            lhsT = ptm[:] if (dgi >= 0 and c == dgi) \
                                else pt[:, (c * 128 - off):(c * 128 - off) + 128]
                            nc.tensor.matmul(pvc(c), lhsT,
                                             vsb[jb][:, h * 129:(h + 1) * 129],
                                             start=(jb == 0 and c % 2 == 0),
                                             stop=(c % 2 == 1 and jb == 4 * g + c),
                                             skip_group_check=True)

                    for jb in range(njb):
                        emit_st(jb)
                        acc[0] += per_jb
                        drain(limit)
                        if len(pend) > LOOKAHEAD:
                            emit_pv()
                    while pend:
                        emit_pv()
                    # normalize group g -> attn slab [i, d] -> DMA transpose
                    slab = wtile([128, 512], bf16, "slab", 3)
                    for c in range(4):
                        rc = wtile([128, 1], f32, "rc", 4)
                        nc.vector.reciprocal(rc[:], pvc(c)[:, 128:129])
                        nc.vector.tensor_scalar_mul(slab[:, c * 128:(c + 1) * 128],
                                                    pvc(c)[:, 0:128], rc[:])
                    nc.sync.dma_start_transpose(
                        attnT[h][:, g * 512:(g + 1) * 512].rearrange("p (c f) -> p c f", c=4),
                        slab[:])
                # flush remaining fillers
                acc[0] = float(len(filler))
                drain(len(filler))

            # ---------------- schedule ----------------
            with tc.tile_pool(name="qkvps", bufs=2, space="PSUM") as qkvps:
                fetch_w(0)
                if H_ > 1:
                    fetch_w(1)
                for u in qkv_closures(0, qkvps):   # slot 0: qkv(h0) alone
                    u()
                for h in range(1, H_):             # slots 1..H-1
                    if h + 1 < H_:
                        fetch_w(h + 1)
                    if h == H_ - 1:
                        for hh in range(H_):
                            fetch_wo(hh)
                    attn_head(h - 1, qkv_closures(h, qkvps))
                if H_ == 1:
                    for hh in range(H_):
                        fetch_wo(hh)

            with tc.tile_pool(name="opsps", bufs=2, space="PSUM") as opsps:
                eng_rr = [nc.scalar.copy, nc.vector.tensor_copy]

                def oproj_unit(sb):
                    stage = wtile([128, Dm_], bf16, "ostage", 2)
                    for db in range(NDB):
                        ps = opsps.tile([128, 512], f32, tag="ops", name="ops")
                        for hh in range(H_):
                            nc.tensor.matmul(ps[:],
                                             attnT[hh][:, sb * 128:(sb + 1) * 128],
                                             wot[hh][:, db * 512:(db + 1) * 512],
                                             start=(hh == 0), stop=(hh == H_ - 1))
                        copy = eng_rr[(sb * NDB + db) % len(eng_rr)]
                        copy(stage[:, db * 512:(db + 1) * 512], ps[:])
                    nc.sync.dma_start(out_d[sb * 128:(sb + 1) * 128, :], stage[:])

                # last head's attention with o_proj filler; o_proj(sb) needs
                # attnT[H_-1] group sb//4 done -> gate fillers per group
                filler = [lambda sb=sb: oproj_unit(sb) for sb in range(SBK)]
                attn_head(H_ - 1, filler, gate=lambda g: 4 * g)

    nc.compile()
    return nc


# --------------------------------------------------------------------------
# host sharding + entry point
# --------------------------------------------------------------------------

def _prep_core_inputs(x, Wq, Wk, Wv, Wo, fp32r=None):
    """Return list of 8 per-core input dicts (bf16, pre-tiled [128, ...])."""
    import ml_dtypes
    bf = ml_dtypes.bfloat16
    perm = _deinterleave_idx()
    DT = D // 128
    in_maps = []
    for c in range(N_CORES):
        b, g = c // GROUPS, c % GROUPS
        heads = range(g * H_PER_CORE, (g + 1) * H_PER_CORE)
        qk_rows = np.concatenate([h * HD + perm for h in heads])
        v_rows = np.concatenate([np.arange(h * HD, (h + 1) * HD) for h in heads])
        wq_t = Wq[qk_rows, :].T.astype(np.float32)   # [D, E]
        wk_t = Wk[qk_rows, :].T.astype(np.float32)
        wv_t = Wv[v_rows, :].T.astype(np.float32)
        wo_t = Wo[:, v_rows].T.astype(np.float32)    # [E, D]

        # x tiles, d-major: xP[p, d*S+s] = x[b][s, d*128+p]
        xb = np.ascontiguousarray(x[b]).astype(bf)               # [S, D]
        xP = xb.T.reshape(DT, 128, S).transpose(1, 0, 2).reshape(128, DT * S)

        def tile_w(w):
            """w [D, C] -> [128, (C/128)*DT*128]: per 128-col block, DT
            d-tiles of [128, 128] laid out d-major."""
            C = w.shape[1]
            nb = C // 128
            out = np.empty((128, nb * DT * 128), dtype=bf)
            wb = w.astype(bf)
            for bi in range(nb):
                t3 = wb[:, bi * 128:(bi + 1) * 128].reshape(DT, 128, 128)
                out[:, bi * DT * 128:(bi + 1) * DT * 128] = (
                    t3.transpose(1, 0, 2).reshape(128, DT * 128))
            return out

        # wqk blocks in (h, p) order: block 2h = q head h, block 2h+1 = k
        wqk = np.empty((D, 2 * E_PER_CORE), dtype=np.float32)
        for h in range(H_PER_CORE):
            wqk[:, (2 * h) * 128:(2 * h + 1) * 128] = wq_t[:, h * 128:(h + 1) * 128]
            wqk[:, (2 * h + 1) * 128:(2 * h + 2) * 128] = wk_t[:, h * 128:(h + 1) * 128]

        woP = wo_t.reshape(H_PER_CORE, 128, D).transpose(1, 0, 2).reshape(
            128, H_PER_CORE * D).astype(bf)

        in_maps.append({
            "xP": np.ascontiguousarray(xP),
            "wqkP": np.ascontiguousarray(tile_w(wqk)),
            "wvP": np.ascontiguousarray(tile_w(wv_t)),
            "woP": np.ascontiguousarray(woP),
        })
    return in_maps


def _is_causal_mask(attn_mask):
    if attn_mask is None:
        return True
    m = np.asarray(attn_mask)
    if m.shape != (1, 1, S, S):
        return False
    m2 = m[0, 0]
    tril = np.tril(np.ones((S, S), dtype=bool))
    return bool(np.all(m2[tril] == 0.0) and np.all(m2[~tril] <= -1.0e30))


def _get_program(mmdt="bf16"):
    key = ("full", "bf16")
    if key not in _CACHE:
        _CACHE[key] = build(S, D, H_PER_CORE)
    return _CACHE[key]


def run_on_hw(in_maps, mmdt="bf16", trace=False, **kwargs):
    """Run the SPMD program on the 8 NeuronCores; returns BassKernelResults."""
    from concourse.bass_utils import run_bass_kernel_spmd
    nc = _get_program(mmdt)
    return run_bass_kernel_spmd(nc, in_maps, core_ids=list(range(N_CORES)),
                                trace=trace, **kwargs)


def kernel(x, Wq, Wk, Wv, Wo, attn_mask=None, **_ignored):
    x = np.asarray(x, dtype=np.float32)
    Wq = np.asarray(Wq, dtype=np.float32)
    Wk = np.asarray(Wk, dtype=np.float32)
    Wv = np.asarray(Wv, dtype=np.float32)
    Wo = np.asarray(Wo, dtype=np.float32)

    if not _is_causal_mask(attn_mask):
        return _np_reference(x, Wq, Wk, Wv, Wo,
                             np.asarray(attn_mask, dtype=np.float32)).astype(np.float32)

    in_maps = _prep_core_inputs(x, Wq, Wk, Wv, Wo)
    res = run_on_hw(in_maps, trace=False)

    out = np.zeros((B, S, D), dtype=np.float32)
    for c in range(N_CORES):
        out[c // GROUPS] += res.results[c]["out"].astype(np.float32)
    return out
